# revision 1
# baseline (speedup 1.0000x reference)
"""Trainium2 Bass kernel: causal attention (QKV projection + causal softmax + AV).

Problem: x[4, 4096, 768] fp32, per-head projections to d=64, full causal
attention per batch, output [4, 4096, 64] fp32.

Sharding: 8 cores = 4 batches x 2 parity groups. Core (b, j) computes the
output rows of batch b whose 128-row block index i satisfies i % 2 == j.
One uniform SPMD program: for j=0 cores the host shifts x down by one
128-row block (prepending zeros), which makes the causal structure of both
parities identical in device coordinates (device q-blocks are always the odd
blocks 1,3,...,31; k-slot g holds true block g-1 for j=0 and g for j=1; the
dead slot 0 of j=0 is zeroed post-exp with a per-core 0/1 scale).

Device pipeline per core (all matmuls bf16, fp32 PSUM accumulation):
  P1 (per 512-row seq chunk): one 3D-output DMA-transpose yields x^T for the
     chunk; two matmul passes with stationary [wq|wq] and [wk|wv] produce
     Q^T (own q-blocks, both partition halves), K^T (low half, SWDGE-
     duplicated to the high half) and V^T (DMA-transposed into V' = [V | 1]).
  P2 (per 512-col q chunk): for consecutive k-slot pairs, two concurrent
     row-tiled matmuls K^T_g.T @ Q^T produce S^T; exp on ACT (scale 1/8,
     causal diagonal masked by a bf16 upper-tri mask, merged across the
     pair's two PSUM banks); AV accumulates V'.T @ P^T into a [65, 512]
     PSUM tile whose row 64 is the softmax denominator (ones column of V').
     The unnormalized [65, 512] tiles go to DRAM; the host divides and
     transposes.
"""

import numpy as np
import ml_dtypes
from contextlib import ExitStack

import concourse.bass as bass
import concourse.mybir as mybir
import concourse.tile as tile
from concourse import bacc
from concourse.bass_utils import run_bass_kernel_spmd

F32 = mybir.dt.float32
BF16 = mybir.dt.bfloat16

SEQ = 4096
DIN = 768
DOUT = 64
NCC = DIN // 128          # 6 contraction chunks
NSC = SEQ // 512          # 8 seq chunks (projection granularity)
NBLK = SEQ // 128         # 32 k-slots
NQC = 4                   # q chunks of 512 local columns (2048 own q rows)
SCALE = 1.0 / 8.0
EXPF = mybir.ActivationFunctionType.Exp

_CACHED_NC = None


def build_nc(dump=False, repeats=1):
    nc = bacc.Bacc("TRN2", target_bir_lowering=False, debug=False)

    x = nc.dram_tensor("x", [SEQ, DIN], BF16, kind="ExternalInput")
    wqq = nc.dram_tensor("wqq", [DIN, 128], BF16, kind="ExternalInput")  # [wq|wq]
    wkv = nc.dram_tensor("wkv", [DIN, 128], BF16, kind="ExternalInput")  # [wv|wk]
    bqq = nc.dram_tensor("bqq", [128, 1], F32, kind="ExternalInput")     # [bq;bq]
    bkv = nc.dram_tensor("bkv", [128, 1], F32, kind="ExternalInput")     # [bk;bv]
    pads = nc.dram_tensor("pads", [128, 1], F32, kind="ExternalInput")   # 1 / 0
    maska = nc.dram_tensor("maska", [128, 512], BF16, kind="ExternalInput")
    idnb = nc.dram_tensor("idnb", [64, 64], BF16, kind="ExternalInput")
    o = nc.dram_tensor("o", [NQC, 65, 512], F32, kind="ExternalOutput")
    if dump:
        okt = nc.dram_tensor("okt", [64, NBLK * 128], BF16, kind="ExternalOutput")
        oqt = nc.dram_tensor("oqt", [128, 16 * 128], BF16, kind="ExternalOutput")
        ovs = nc.dram_tensor("ovs", [128, NBLK * 65], BF16, kind="ExternalOutput")
        opt = nc.dram_tensor("opt", [128, 1024], BF16, kind="ExternalOutput")

    with tile.TileContext(nc) as tc, ExitStack() as ctx:
        cpool = ctx.enter_context(tc.tile_pool(name="const", bufs=1))
        vtp = ctx.enter_context(tc.tile_pool(name="vt", bufs=2))
        ptp = ctx.enter_context(tc.tile_pool(name="pt", bufs=3))
        ocp = ctx.enter_context(tc.tile_pool(name="oc", bufs=2))
        psproj = ctx.enter_context(tc.tile_pool(name="psproj", bufs=2, space="PSUM"))
        psst = ctx.enter_context(tc.tile_pool(name="psst", bufs=2, space="PSUM"))
        psav = ctx.enter_context(tc.tile_pool(name="psav", bufs=2, space="PSUM"))

        wqq_sb = cpool.tile([128, NCC * 128], BF16)
        wkv_sb = cpool.tile([128, NCC * 128], BF16)
        bqq_sb = cpool.tile([128, 1], F32)
        bkv_sb = cpool.tile([128, 1], F32)
        pads_sb = cpool.tile([128, 1], F32)
        mask_sb = cpool.tile([128, 512], BF16)
        idn_sb = cpool.tile([64, 64], BF16)
        kt2 = cpool.tile([128, NBLK * 128], BF16)   # K^T, both partition halves
        xtf = cpool.tile([128, NSC * NCC * 512], BF16)  # x^T, whole sequence
        qt = cpool.tile([128, 16 * 128], BF16)      # Q^T own blocks, both halves
        vs = cpool.tile([128, NBLK * 65], BF16)     # V' = [V | 1] per k-slot

        # weights laid out [c-in-chunk partition, (chunk, out_col) free]
        nc.sync.dma_start(
            wqq_sb[:].rearrange("p (cc m) -> p cc m", cc=NCC),
            wqq.rearrange("(cc p) m -> p cc m", p=128),
        )
        nc.sync.dma_start(
            wkv_sb[:].rearrange("p (cc m) -> p cc m", cc=NCC),
            wkv.rearrange("(cc p) m -> p cc m", p=128),
        )
        nc.sync.dma_start(bqq_sb[:], bqq[:, :])
        nc.sync.dma_start(bkv_sb[:], bkv[:, :])
        nc.sync.dma_start(pads_sb[:], pads[:, :])
        nc.sync.dma_start(mask_sb[:], maska[:, :])
        nc.sync.dma_start(idn_sb[:], idnb[:, :])
        # ones column of V'
        nc.vector.memset(
            vs[:].rearrange("p (g e) -> p g e", g=NBLK)[:, :, 64:65], 1.0
        )

        def xts(sc, cc):
            base = sc * NCC * 512 + cc * 512
            return xtf[:, base:base + 512]

        def trans_chunk(sc):
            """DMA-transpose x rows [sc*512, (sc+1)*512) into resident x^T."""
            nc.sync.dma_start_transpose(
                xtf[:, sc * NCC * 512:(sc + 1) * NCC * 512]
                .rearrange("p (cc s) -> p cc s", cc=NCC),
                x[sc * 512:(sc + 1) * 512, :],
            )

        def passA_chunk(sc):
            """Q^T for own (odd) q-blocks of this chunk, [wq|wq] stationary."""
            qp = psproj.tile([128, 256], F32, tag="proj")
            for cc in range(NCC):
                rhs = (
                    xts(sc, cc)
                    .rearrange("p (a b s) -> p a b s", a=2, b=2)[:, :, 1, :]
                )
                nc.tensor.matmul(
                    qp[:], wqq_sb[:, cc * 128:(cc + 1) * 128], rhs,
                    start=(cc == 0), stop=(cc == NCC - 1),
                )
            nc.vector.tensor_scalar_add(
                qt[:, sc * 256:(sc + 1) * 256], qp[:], bqq_sb[:]
            )

        def passB_chunk(sc):
            """K^T (rows 64-127) and V^T (rows 0-63), [wv|wk] stationary."""
            kp = psproj.tile([128, 512], F32, tag="proj")
            for cc in range(NCC):
                nc.tensor.matmul(
                    kp[:], wkv_sb[:, cc * 128:(cc + 1) * 128],
                    xts(sc, cc),
                    start=(cc == 0), stop=(cc == NCC - 1),
                )
            nc.vector.tensor_scalar_add(
                kt2[64:128, sc * 512:(sc + 1) * 512], kp[64:128, :], bkv_sb[64:128, :]
            )
            hi = kt2[64:128, sc * 512:(sc + 1) * 512].rearrange(
                "p (a b s) -> p a b s", a=2, b=2)[:, :, 0, :]
            lo = kt2[0:64, sc * 512:(sc + 1) * 512].rearrange(
                "p (a b s) -> p a b s", a=2, b=2)[:, :, 0, :]
            nc.gpsimd.dma_start(lo, hi)
            vt = vtp.tile([128, 512], BF16)
            nc.vector.tensor_scalar_add(
                vt[0:64, :], kp[0:64, :], bkv_sb[0:64, :]
            )
            # V' blocks via PE transpose (DMA-transpose is only HW-exact for
            # the whole-row DRAM-sourced x case)
            vp = psproj.tile([128, 256], BF16, tag="proj")
            for t in range(4):
                nc.tensor.transpose(
                    vp[:, t * 64:(t + 1) * 64],
                    vt[0:64, t * 128:(t + 1) * 128],
                    idn_sb[:],
                )
            nc.vector.tensor_copy(
                vs[:].rearrange("p (g e) -> p g e", g=NBLK)[
                    :, sc * 4:(sc + 1) * 4, 0:64
                ],
                vp[:].rearrange("p (g e) -> p g e", g=4),
            )

        parts = {}

        def attn_seg(c, p_lo, p_hi, final):
            """Attention pairs [p_lo, p_hi) for local q cols [c*512, (c+1)*512)."""
            npairs = 4 * c + 4           # k-slots 0..8c+7 in consecutive pairs
            av = psav.tile([65, 512], F32, tag="av")
            first_av = [True]

            def slot_geom(g):
                s = g - (8 * c + 1)
                if s < 1:
                    return 0, 512
                off = 128 * ((s + 1) // 2)
                return off, 512 - off

            for p in range(p_lo, p_hi):
                g0, g1 = 2 * p, 2 * p + 1
                off0, w0 = slot_geom(g0)
                off1, w1 = slot_geom(g1)
                st = psst.tile([128, 1024], F32, tag="st")
                nc.tensor.matmul(
                    st[:, 0:w0], kt2[0:64, g0 * 128:(g0 + 1) * 128],
                    qt[0:64, c * 512 + off0: c * 512 + off0 + w0],
                    start=True, stop=True, tile_position=(0, 0),
                )
                nc.tensor.matmul(
                    st[:, 512:512 + w1], kt2[64:128, g1 * 128:(g1 + 1) * 128],
                    qt[64:128, c * 512 + off1: c * 512 + off1 + w1],
                    start=True, stop=True, tile_position=(64, 0),
                )
                pt = ptp.tile([128, 1024], BF16)
                if w0 < 512:
                    # exact-width exps: skip the gap instead of memset+merge
                    nc.scalar.activation(pt[:, 0:w0], st[:, 0:w0],
                                         EXPF, bias=0.0, scale=SCALE)
                    nc.scalar.activation(pt[:, 512:512 + w1], st[:, 512:512 + w1],
                                         EXPF, bias=0.0, scale=SCALE)
                else:
                    nc.scalar.activation(pt[:, 0:512 + w1], st[:, 0:512 + w1],
                                         EXPF, bias=0.0, scale=SCALE)
                if p == 0:
                    # kill the j=0 dead slot 0 (pads = 0 there, 1 for j=1)
                    nc.vector.tensor_scalar_mul(
                        pt[:, 0:512], pt[:, 0:512], pads_sb[:]
                    )
                if p >= npairs - 4:
                    # odd member of the last four pairs is causal-diagonal
                    nc.vector.tensor_mul(
                        pt[:, 512:512 + w1], pt[:, 512:512 + w1], mask_sb[:, 0:w1]
                    )
                if dump and c == 0 and p == 0:
                    nc.sync.dma_start(opt[:, :], pt[:])
                nc.tensor.matmul(
                    av[:, off0:off0 + w0], vs[:, g0 * 65:(g0 + 1) * 65],
                    pt[:, 0:w0],
                    start=first_av[0], stop=False,
                )
                first_av[0] = False
                nc.tensor.matmul(
                    av[:, off1:off1 + w1], vs[:, g1 * 65:(g1 + 1) * 65],
                    pt[:, 512:512 + w1],
                    start=False, stop=(p == p_hi - 1),
                )
            if final:
                oc = ocp.tile([65, 512], F32)
                if c in parts:
                    nc.vector.tensor_add(oc[:], av[:], parts.pop(c)[:])
                else:
                    nc.vector.tensor_copy(oc[:], av[:])
                nc.gpsimd.dma_start(o[c, :, :], oc[:])
            else:
                part = ocp.tile([65, 512], F32, tag="part")
                nc.vector.tensor_copy(part[:], av[:])
                parts[c] = part

        # chunk 3's first attention half only needs k-slots 0-15 plus its own
        # Q columns (sc6/7): with x^T fully resident, project that Q early so
        # the exp load isn't all at the tail
        for _rep in range(repeats):
            for sc in range(NSC):
                trans_chunk(sc)
            passA_chunk(0)
            passB_chunk(0)
            passA_chunk(1)
            passB_chunk(1)
            attn_seg(0, 0, 4, True)
            passA_chunk(2)
            passB_chunk(2)
            passA_chunk(3)
            passB_chunk(3)
            attn_seg(1, 0, 8, True)
            passA_chunk(6)
            passA_chunk(7)
            attn_seg(3, 0, 8, False)
            passA_chunk(4)
            passB_chunk(4)
            passA_chunk(5)
            passB_chunk(5)
            attn_seg(2, 0, 12, True)
            passB_chunk(6)
            passB_chunk(7)
            attn_seg(3, 8, 16, True)
        if dump:
            nc.sync.dma_start(okt[:, :], kt2[64:128, :])
            nc.sync.dma_start(oqt[:, :], qt[:])
            nc.sync.dma_start(ovs[:, :], vs[:])

    nc.compile()
    return nc


def _get_nc():
    global _CACHED_NC
    if _CACHED_NC is None:
        _CACHED_NC = build_nc()
    return _CACHED_NC


def _host_inputs(x, wq, bq, wk, bk, wv, bv):
    bf = ml_dtypes.bfloat16
    wqq = np.concatenate([wq, wq], axis=1).astype(bf)
    wkv = np.concatenate([wv, wk], axis=1).astype(bf)
    bqq = np.concatenate([bq, bq])[:, None].astype(np.float32)
    bkv = np.concatenate([bv, bk])[:, None].astype(np.float32)
    tri = np.triu(np.ones((128, 128), np.float32))
    maska = np.concatenate([tri, np.ones((128, 384), np.float32)], axis=1).astype(bf)
    idnb = np.eye(64, dtype=np.float32).astype(bf)
    xbf = np.ascontiguousarray(x).astype(bf)

    in_maps = []
    for core in range(8):
        b, j = core // 2, core % 2
        if j == 0:
            xdev = np.concatenate(
                [np.zeros((128, DIN), bf), xbf[b][: SEQ - 128]], axis=0
            )
            ps = np.zeros((128, 1), np.float32)
        else:
            xdev = xbf[b]
            ps = np.ones((128, 1), np.float32)
        in_maps.append({
            "x": np.ascontiguousarray(xdev),
            "wqq": wqq, "wkv": wkv, "bqq": bqq, "bkv": bkv,
            "pads": ps, "maska": maska, "idnb": idnb,
        })
    return in_maps


def _assemble(results):
    out = np.empty((4, SEQ, DOUT), np.float32)
    for core in range(8):
        b, j = core // 2, core % 2
        od = results[core]["o"]  # [NQC, 65, 512]
        for c in range(NQC):
            num = od[c, 0:64, :].astype(np.float64)
            den = od[c, 64, :].astype(np.float64)
            oc = (num / den).T.astype(np.float32)  # [512, 64]
            for t in range(4):
                r0 = (8 * c + 2 * t + j) * 128
                out[b, r0:r0 + 128] = oc[t * 128:(t + 1) * 128]
    return out


def kernel(x, wq, bq, wk, bk, wv, bv):
    x = np.asarray(x, dtype=np.float32)
    args = [np.asarray(a, dtype=np.float32) for a in (wq, bq, wk, bk, wv, bv)]
    nc = _get_nc()
    in_maps = _host_inputs(x, *args)
    br = run_bass_kernel_spmd(nc, in_maps, core_ids=list(range(8)))
    return _assemble(br.results)



# revision 3
# speedup vs baseline: 1.3271x; 1.3271x over previous
"""Trainium2 Bass kernel: causal attention (QKV projection + causal softmax + AV).

Problem: x[4, 4096, 768] fp32, per-head projections to d=64, full causal
attention per batch, output [4, 4096, 64] fp32.

Sharding: 8 cores = 4 batches x 2 parity groups. Core (b, j) computes the
output rows of batch b whose 128-row block index i satisfies i % 2 == j.
One uniform SPMD program: for j=0 cores the host shifts x down by one
128-row block (prepending zeros), which makes the causal structure of both
parities identical in device coordinates (device q-blocks are always the odd
blocks 1,3,...,31; k-slot g holds true block g-1 for j=0 and g for j=1; the
dead slot 0 of j=0 is neutralized by zeroing V' slot 0, so its exp(0)=1
weights contribute nothing to numerator or denominator).

Device pipeline per core:
  x^T arrives host-pre-transposed (plain DMA loads, no DMA-transpose).
  Projections per 512-row chunk: stationary [wq] produces Q^T on PSUM
  partitions 64-127; stationary [wv|wk] produces V^T (0-63) and K^T (64-127).
  Q^T (+bq) and K^T (bk dropped: softmax is invariant to score offsets that
  are constant along k) are written as fp8e4 into zero-padded DoubleRow
  buffers [64, 2, cols] whose second contraction-tile group is zeroed once.
  Scores: one fp8 DoubleRow matmul per k-slot (0.5 cycles/row), packed
  tail/head-aligned around the PSUM bank boundary so each slot pair is a
  single exact-width exp on ACT. AV accumulates V'.T @ P^T in bf16 into a
  [65, 512] PSUM tile whose row 64 is the softmax denominator. Host divides
  and transposes.
"""

import numpy as np
import ml_dtypes
from contextlib import ExitStack

import concourse.bass as bass
import concourse.mybir as mybir
import concourse.tile as tile
from concourse import bacc
from concourse.bass_utils import run_bass_kernel_spmd

F32 = mybir.dt.float32
BF16 = mybir.dt.bfloat16
FP8 = mybir.dt.float8e4

SEQ = 4096
DIN = 768
DOUT = 64
NCC = DIN // 128          # 6 contraction chunks
NSC = SEQ // 512          # 8 seq chunks (projection granularity)
NBLK = SEQ // 128         # 32 k-slots
NQC = 4                   # q chunks of 512 local columns (2048 own q rows)
SCALE = 1.0 / 8.0
EXPF = mybir.ActivationFunctionType.Exp
DR = mybir.MatmulPerfMode.DoubleRow

_CACHED_NC = None


def build_nc(dump=False, repeats=1):
    nc = bacc.Bacc("TRN2", target_bir_lowering=False, debug=False)

    xt = nc.dram_tensor("xt", [DIN, SEQ], BF16, kind="ExternalInput")
    wq = nc.dram_tensor("wq", [DIN, 64], BF16, kind="ExternalInput")
    wkv = nc.dram_tensor("wkv", [DIN, 128], BF16, kind="ExternalInput")  # [wv|wk]
    bq = nc.dram_tensor("bq", [64, 1], F32, kind="ExternalInput")
    bv = nc.dram_tensor("bv", [64, 1], F32, kind="ExternalInput")
    pads = nc.dram_tensor("pads", [128, 1], F32, kind="ExternalInput")   # 1 / 0
    maska = nc.dram_tensor("maska", [128, 512], BF16, kind="ExternalInput")
    idnb = nc.dram_tensor("idnb", [64, 64], BF16, kind="ExternalInput")
    o = nc.dram_tensor("o", [NQC, 65, 512], F32, kind="ExternalOutput")
    if dump:
        okt = nc.dram_tensor("okt", [64, NBLK * 128], BF16, kind="ExternalOutput")
        oqt = nc.dram_tensor("oqt", [64, 16 * 128], BF16, kind="ExternalOutput")
        ovs = nc.dram_tensor("ovs", [128, NBLK * 65], BF16, kind="ExternalOutput")
        opt = nc.dram_tensor("opt", [128, 1024], BF16, kind="ExternalOutput")

    with tile.TileContext(nc) as tc, ExitStack() as ctx:
        cpool = ctx.enter_context(tc.tile_pool(name="const", bufs=1))
        vtp = ctx.enter_context(tc.tile_pool(name="vt", bufs=2))
        ptp = ctx.enter_context(tc.tile_pool(name="pt", bufs=3))
        ocp = ctx.enter_context(tc.tile_pool(name="oc", bufs=2))
        psproj = ctx.enter_context(tc.tile_pool(name="psproj", bufs=2, space="PSUM"))
        psst = ctx.enter_context(tc.tile_pool(name="psst", bufs=2, space="PSUM"))
        psav = ctx.enter_context(tc.tile_pool(name="psav", bufs=2, space="PSUM"))

        wq_sb = cpool.tile([128, NCC * 64], BF16)
        wkv_sb = cpool.tile([128, NCC * 128], BF16)
        bq_sb = cpool.tile([128, 1], F32)     # rows 64:128 hold bq
        bv_sb = cpool.tile([64, 1], F32)
        pads_sb = cpool.tile([128, 1], F32)
        mask_sb = cpool.tile([128, 512], BF16)
        idn_sb = cpool.tile([64, 64], BF16)
        xtf = cpool.tile([128, NSC * NCC * 512], BF16)  # x^T, [p, (sc, cc, s)]
        qdr = cpool.tile([128, 2 * 2048], FP8)  # Q^T fp8, rows 64:128, [(i, q)]
        kdr = cpool.tile([128, 2 * 4096], FP8)  # K^T fp8, rows 64:128, [(i, m)]
        vs = cpool.tile([128, NBLK * 65], BF16)  # V' = [V | 1] per k-slot

        # weights laid out [c-in-chunk partition, (chunk, out_col) free]
        nc.sync.dma_start(
            wq_sb[:].rearrange("p (cc m) -> p cc m", cc=NCC),
            wq.rearrange("(cc p) m -> p cc m", p=128),
        )
        nc.sync.dma_start(
            wkv_sb[:].rearrange("p (cc m) -> p cc m", cc=NCC),
            wkv.rearrange("(cc p) m -> p cc m", p=128),
        )

        def loadx(sc):
            nc.sync.dma_start(
                xtf[:, sc * NCC * 512:(sc + 1) * NCC * 512]
                .rearrange("p (cc s) -> p cc s", cc=NCC),
                xt.rearrange("(cc p) s -> p cc s", p=128)[
                    :, :, sc * 512:(sc + 1) * 512
                ],
            )

        loadx(0)
        nc.sync.dma_start(bq_sb[64:128, :], bq[:, :])
        nc.sync.dma_start(bv_sb[:], bv[:, :])
        nc.sync.dma_start(pads_sb[:], pads[:, :])
        nc.sync.dma_start(idn_sb[:], idnb[:, :])
        loadx(1)
        nc.sync.dma_start(mask_sb[:], maska[:, :])
        for sc in range(2, NSC):
            loadx(sc)

        # ones column of V'
        nc.vector.memset(
            vs[:].rearrange("p (g e) -> p g e", g=NBLK)[:, :, 64:65], 1.0
        )
        # zero the second DoubleRow contraction-tile group (also guards
        # against NaN garbage multiplying the other side's zeros)
        nc.gpsimd.memset(qdr[64:128, 2048:2048 + 512], 0.0)
        nc.gpsimd.memset(kdr[64:128, 4096:4096 + 1024], 0.0)
        nc.gpsimd.memset(qdr[64:128, 2048 + 512:4096], 0.0)
        nc.gpsimd.memset(kdr[64:128, 4096 + 1024:8192], 0.0)

        def xts(sc, cc):
            base = sc * NCC * 512 + cc * 512
            return xtf[:, base:base + 512]

        def passA(sc):
            """Q^T for own (odd) q-blocks of this chunk, fp8 into qdr."""
            qp = psproj.tile([128, 256], F32, tag="proj")
            for cc in range(NCC):
                rhs = (
                    xts(sc, cc)
                    .rearrange("p (a b s) -> p a b s", a=2, b=2)[:, :, 1, :]
                )
                nc.tensor.matmul(
                    qp[64:128, :], wq_sb[:, cc * 64:(cc + 1) * 64], rhs,
                    start=(cc == 0), stop=(cc == NCC - 1),
                )
            nc.vector.tensor_scalar_add(
                qdr[64:128, :].rearrange("p (i q) -> p i q", i=2)[
                    :, 0, sc * 256:(sc + 1) * 256
                ],
                qp[64:128, :], bq_sb[64:128, :],
            )

        def passB(sc):
            """K^T (fp8, no bias) and V' from [wv|wk] stationary."""
            kvp = psproj.tile([128, 512], F32, tag="proj")
            for cc in range(NCC):
                nc.tensor.matmul(
                    kvp[:], wkv_sb[:, cc * 128:(cc + 1) * 128],
                    xts(sc, cc),
                    start=(cc == 0), stop=(cc == NCC - 1),
                )
            nc.vector.tensor_copy(
                kdr[64:128, :].rearrange("p (i m) -> p i m", i=2)[
                    :, 0, sc * 512:(sc + 1) * 512
                ],
                kvp[64:128, :],
            )
            vt = vtp.tile([128, 512], BF16)
            nc.vector.tensor_scalar_add(
                vt[0:64, :], kvp[0:64, :], bv_sb[:, :]
            )
            vp = psproj.tile([128, 256], BF16, tag="proj")
            for t in range(4):
                nc.tensor.transpose(
                    vp[:, t * 64:(t + 1) * 64],
                    vt[0:64, t * 128:(t + 1) * 128],
                    idn_sb[:],
                )
            nc.vector.tensor_copy(
                vs[:].rearrange("p (g e) -> p g e", g=NBLK)[
                    :, sc * 4:(sc + 1) * 4, 0:64
                ],
                vp[:].rearrange("p (g e) -> p g e", g=4),
            )
            if sc == 0:
                # neutralize the j=0 dead slot 0 (pads = 0 there, 1 for j=1)
                nc.vector.tensor_scalar_mul(
                    vs[:, 0:65], vs[:, 0:65], pads_sb[:]
                )

        def kslot(g):
            return kdr[64:128, :].rearrange("p (i m) -> p i m", i=2)[
                :, :, g * 128:(g + 1) * 128
            ]

        def qsl(c, off, w):
            return qdr[64:128, :].rearrange("p (i q) -> p i q", i=2)[
                :, :, c * 512 + off:c * 512 + off + w
            ]

        def slot_geom(c, g):
            s = g - (8 * c + 1)
            if s < 1:
                return 0, 512
            off = 128 * ((s + 1) // 2)
            return off, 512 - off

        def attn_chunk(c, fillers):
            """All attention pairs of q-chunk c; fillers = [(after_pair,
            fn), ...] projection work interleaved into the PE stream."""
            npairs = 4 * c + 4
            av = psav.tile([65, 512], F32, tag="av")
            pend = None

            def emit_av(args):
                av_, off0, w0, off1, w1, pt_, g0, g1, first, last = args
                nc.tensor.matmul(
                    av_[:, off0:off0 + w0], vs[:, g0 * 65:(g0 + 1) * 65],
                    pt_[:, 512 - w0:512], start=first, stop=False,
                )
                nc.tensor.matmul(
                    av_[:, off1:off1 + w1], vs[:, g1 * 65:(g1 + 1) * 65],
                    pt_[:, 512:512 + w1], start=False, stop=last,
                )

            fill = dict(fillers)
            for p in range(npairs):
                g0, g1 = 2 * p, 2 * p + 1
                off0, w0 = slot_geom(c, g0)
                off1, w1 = slot_geom(c, g1)
                lo0 = 512 - w0
                st = psst.tile([128, 1024], F32, tag="st")
                nc.tensor.matmul(
                    st[:, lo0:512], kslot(g0), qsl(c, off0, w0),
                    start=True, stop=True, perf_mode=DR,
                )
                nc.tensor.matmul(
                    st[:, 512:512 + w1], kslot(g1), qsl(c, off1, w1),
                    start=True, stop=True, perf_mode=DR,
                )
                if p in fill:
                    fill[p]()
                if pend is not None:
                    emit_av(pend)
                pt = ptp.tile([128, 1024], BF16)
                nc.scalar.activation(pt[:, lo0:512 + w1], st[:, lo0:512 + w1],
                                     EXPF, bias=0.0, scale=SCALE)
                if p >= npairs - 4:
                    # odd member of the last four pairs is causal-diagonal
                    nc.vector.tensor_mul(
                        pt[:, 512:512 + w1], pt[:, 512:512 + w1],
                        mask_sb[:, 0:w1]
                    )
                if dump and c == 0 and p == 0:
                    nc.sync.dma_start(opt[:, :], pt[:])
                pend = (av, off0, w0, off1, w1, pt, g0, g1,
                        p == 0, p == npairs - 1)
            emit_av(pend)
            oc = ocp.tile([65, 512], F32)
            nc.vector.tensor_copy(oc[:], av[:])
            nc.sync.dma_start(o[c, :, :], oc[:])

        for _rep in range(repeats):
            passA(0)
            passB(0)
            passA(1)
            attn_chunk(0, [
                (0, lambda: passB(1)),
                (1, lambda: passA(2)),
                (2, lambda: passB(2)),
                (3, lambda: passA(3)),
            ])
            attn_chunk(1, [
                (1, lambda: passB(3)),
                (3, lambda: passA(4)),
                (4, lambda: passB(4)),
                (5, lambda: passA(5)),
                (6, lambda: passB(5)),
            ])
            attn_chunk(2, [
                (2, lambda: passA(6)),
                (4, lambda: passB(6)),
                (6, lambda: passA(7)),
                (8, lambda: passB(7)),
            ])
            attn_chunk(3, [])
        if dump:
            nc.sync.dma_start(
                okt[:, :],
                kdr[64:128, :].rearrange("p (i m) -> p i m", i=2)[:, 0, :],
            )
            nc.sync.dma_start(
                oqt[:, :],
                qdr[64:128, :].rearrange("p (i q) -> p i q", i=2)[:, 0, :],
            )
            nc.sync.dma_start(ovs[:, :], vs[:])

    nc.compile()
    return nc


def _get_nc():
    global _CACHED_NC
    if _CACHED_NC is None:
        _CACHED_NC = build_nc()
    return _CACHED_NC


def _host_inputs(x, wq, bq, wk, bk, wv, bv):
    bf = ml_dtypes.bfloat16
    wqb = np.ascontiguousarray(wq).astype(bf)
    wkv = np.concatenate([wv, wk], axis=1).astype(bf)
    bqc = bq[:, None].astype(np.float32)
    bvc = bv[:, None].astype(np.float32)
    tri = np.triu(np.ones((128, 128), np.float32))
    maska = np.concatenate([tri, np.ones((128, 384), np.float32)], axis=1).astype(bf)
    idnb = np.eye(64, dtype=np.float32).astype(bf)
    xbf = np.ascontiguousarray(x).astype(bf)

    in_maps = []
    for core in range(8):
        b, j = core // 2, core % 2
        if j == 0:
            xdev = np.concatenate(
                [np.zeros((128, DIN), bf), xbf[b][: SEQ - 128]], axis=0
            )
            ps = np.zeros((128, 1), np.float32)
        else:
            xdev = xbf[b]
            ps = np.ones((128, 1), np.float32)
        in_maps.append({
            "xt": np.ascontiguousarray(xdev.T),
            "wq": wqb, "wkv": wkv, "bq": bqc, "bv": bvc,
            "pads": ps, "maska": maska, "idnb": idnb,
        })
    return in_maps


def _assemble(results):
    out = np.empty((4, SEQ, DOUT), np.float32)
    for core in range(8):
        b, j = core // 2, core % 2
        od = results[core]["o"]  # [NQC, 65, 512]
        for c in range(NQC):
            num = od[c, 0:64, :].astype(np.float64)
            den = od[c, 64, :].astype(np.float64)
            oc = (num / den).T.astype(np.float32)  # [512, 64]
            for t in range(4):
                r0 = (8 * c + 2 * t + j) * 128
                out[b, r0:r0 + 128] = oc[t * 128:(t + 1) * 128]
    return out


def kernel(x, wq, bq, wk, bk, wv, bv):
    x = np.asarray(x, dtype=np.float32)
    args = [np.asarray(a, dtype=np.float32) for a in (wq, bq, wk, bk, wv, bv)]
    nc = _get_nc()
    in_maps = _host_inputs(x, *args)
    br = run_bass_kernel_spmd(nc, in_maps, core_ids=list(range(8)))
    return _assemble(br.results)


# revision 6
# speedup vs baseline: 1.3781x; 1.0384x over previous
"""Trainium2 Bass kernel: causal attention (QKV projection + causal softmax + AV).

Problem: x[4, 4096, 768] fp32, per-head projections to d=64, full causal
attention per batch, output [4, 4096, 64] fp32.

Sharding: 8 cores = 4 batches x 2 parity groups. Core (b, j) computes the
output rows of batch b whose 128-row block index i satisfies i % 2 == j.
One uniform SPMD program: for j=0 cores the host shifts x down by one
128-row block (prepending zeros), which makes the causal structure of both
parities identical in device coordinates (device q-blocks are always the odd
blocks 1,3,...,31; k-slot g holds true block g-1 for j=0 and g for j=1; the
dead slot 0 of j=0 is neutralized by zeroing V' slot 0, so its exp(0)=1
weights contribute nothing to numerator or denominator).

Device pipeline per core:
  x^T arrives host-pre-transposed (plain DMA loads, no DMA-transpose).
  A short stream of dummy matmuls at t=0 keeps the tensor engine
  continuously busy so its p-state clock is fully ramped when real
  projections start.
  Projections per 512-row chunk: stationary [wq] produces Q^T on PSUM
  partitions 64-127; stationary [wv|wk] produces V^T (0-63) and K^T (64-127).
  Q^T (+bq) and K^T (bk dropped: softmax is invariant to score offsets that
  are constant along k) are written as fp8e4 into zero-padded DoubleRow
  buffers [64, 2, cols] whose second contraction-tile group is zeroed once.
  Scores: one fp8 DoubleRow matmul per k-slot (0.5 cycles/row). The two
  slots of a pair always share the same width and column range, so AV is a
  single DoubleRow matmul per pair over fp8 P and fp8 V' (slot pair as the
  two contraction-tile groups), accumulating into a [65, 512] PSUM tile
  whose row 64 is the softmax denominator. Chunk-0 pair-0 (rows that attend
  very few keys, where fp8 V error would not average out) uses bf16 P/V'.
  Host divides and transposes.
"""

import numpy as np
import ml_dtypes
from contextlib import ExitStack

import concourse.bass as bass
import concourse.mybir as mybir
import concourse.tile as tile
from concourse import bacc
from concourse.bass_utils import run_bass_kernel_spmd

F32 = mybir.dt.float32
BF16 = mybir.dt.bfloat16
FP8 = mybir.dt.float8e4

SEQ = 4096
DIN = 768
DOUT = 64
NCC = DIN // 128          # 6 contraction chunks
NSC = SEQ // 512          # 8 seq chunks (projection granularity)
NBLK = SEQ // 128         # 32 k-slots
NQC = 4                   # q chunks of 512 local columns (2048 own q rows)
NWARM = 16                # PE p-state warmup matmuls
SCALE = 1.0 / 8.0
EXPF = mybir.ActivationFunctionType.Exp
DR = mybir.MatmulPerfMode.DoubleRow

_CACHED_NC = None


def build_nc(dump=False, repeats=1):
    nc = bacc.Bacc("TRN2", target_bir_lowering=False, debug=False)

    xt = nc.dram_tensor("xt", [DIN, SEQ], BF16, kind="ExternalInput")
    wqr = nc.dram_tensor("wqr", [128, NCC * 64], BF16, kind="ExternalInput")
    wkvr = nc.dram_tensor("wkvr", [128, NCC * 128], BF16, kind="ExternalInput")
    bq = nc.dram_tensor("bq", [64, 1], F32, kind="ExternalInput")
    bv = nc.dram_tensor("bv", [64, 1], F32, kind="ExternalInput")
    pads = nc.dram_tensor("pads", [128, 1], F32, kind="ExternalInput")   # 1 / 0
    maska = nc.dram_tensor("maska", [128, 512], BF16, kind="ExternalInput")
    idnb = nc.dram_tensor("idnb", [64, 64], BF16, kind="ExternalInput")
    o = nc.dram_tensor("o", [NQC, 65, 512], F32, kind="ExternalOutput")
    if dump:
        okt = nc.dram_tensor("okt", [64, NBLK * 128], FP8, kind="ExternalOutput")
        oqt = nc.dram_tensor("oqt", [64, 16 * 128], FP8, kind="ExternalOutput")
        ovs = nc.dram_tensor("ovs", [128, NBLK * 80], FP8, kind="ExternalOutput")

    with tile.TileContext(nc) as tc, ExitStack() as ctx:
        cpool = ctx.enter_context(tc.tile_pool(name="const", bufs=1))
        vtp = ctx.enter_context(tc.tile_pool(name="vt", bufs=2))
        ptp = ctx.enter_context(tc.tile_pool(name="pt", bufs=3))
        ocp = ctx.enter_context(tc.tile_pool(name="oc", bufs=2))
        psproj = ctx.enter_context(tc.tile_pool(name="psproj", bufs=2, space="PSUM"))
        psst = ctx.enter_context(tc.tile_pool(name="psst", bufs=2, space="PSUM"))
        psav = ctx.enter_context(tc.tile_pool(name="psav", bufs=2, space="PSUM"))

        wq_sb = cpool.tile([128, NCC * 64], BF16)
        wkv_sb = cpool.tile([128, NCC * 128], BF16)
        bq_sb = cpool.tile([128, 1], F32)     # rows 64:128 hold bq
        bv_sb = cpool.tile([64, 1], F32)
        pads_sb = cpool.tile([128, 1], F32)
        mask_sb = cpool.tile([128, 512], BF16)
        idn_sb = cpool.tile([64, 64], BF16)
        warm = cpool.tile([128, 256], BF16)
        xtf = cpool.tile([128, NSC * NCC * 512], BF16)  # x^T, [p, (sc, cc, s)]
        qdr = cpool.tile([128, 2 * 2048], FP8)  # Q^T fp8, rows 64:128, [(i, q)]
        kdr = cpool.tile([128, 2 * 4096], FP8)  # K^T fp8, rows 64:128, [(i, m)]
        vs = cpool.tile([128, NBLK * 80], FP8)  # V' = [V | 1 | 0pad] per k-slot
        # (80-wide slots: dual-fp8 ldweights needs a 16-byte-aligned
        # stride between the two contraction-tile groups)
        vsb = cpool.tile([128, 2 * 65], BF16)   # bf16 V' for slots 0,1

        # PE p-state warmup: dummy matmuls on a zeroed tile keep the tensor
        # engine busy from t~0 so the clock is ramped when real work arrives
        nc.vector.memset(warm[:], 0.0)
        for _ in range(NWARM):
            wp = psproj.tile([128, 256], F32, tag="proj")
            nc.tensor.matmul(wp[:], warm[:, 0:128], warm[:], start=True, stop=True)

        def loadx(sc):
            nc.sync.dma_start(
                xtf[:, sc * NCC * 512:(sc + 1) * NCC * 512]
                .rearrange("p (cc s) -> p cc s", cc=NCC),
                xt.rearrange("(cc p) s -> p cc s", p=128)[
                    :, :, sc * 512:(sc + 1) * 512
                ],
            )

        loadx(0)
        nc.sync.dma_start(wq_sb[:], wqr[:, :])
        nc.sync.dma_start(wkv_sb[:], wkvr[:, :])
        loadx(1)
        nc.sync.dma_start(bq_sb[64:128, :], bq[:, :])
        nc.sync.dma_start(bv_sb[:], bv[:, :])
        nc.sync.dma_start(pads_sb[:], pads[:, :])
        nc.sync.dma_start(idn_sb[:], idnb[:, :])
        nc.sync.dma_start(mask_sb[:], maska[:, :])
        for sc in range(2, NSC):
            loadx(sc)

        # ones column of V'
        nc.vector.memset(
            vs[:].rearrange("p (g e) -> p g e", g=NBLK)[:, :, 64:65], 1.0
        )
        nc.vector.memset(
            vs[:].rearrange("p (g e) -> p g e", g=NBLK)[:, :, 65:80], 0.0
        )
        nc.vector.memset(
            vsb[:].rearrange("p (g e) -> p g e", g=2)[:, :, 64:65], 1.0
        )
        # zero the second DoubleRow contraction-tile group of Q^T/K^T (both
        # sides, guarding against NaN garbage multiplying the other's zeros);
        # ordered so the regions attention chunk 0 needs are zeroed first
        nc.gpsimd.memset(qdr[64:128, 2048:2048 + 512], 0.0)
        nc.gpsimd.memset(kdr[64:128, 4096:4096 + 1024], 0.0)
        nc.gpsimd.memset(kdr[64:128, 4096 + 1024:8192], 0.0)
        nc.gpsimd.memset(qdr[64:128, 2048 + 512:4096], 0.0)

        def xts(sc, cc):
            base = sc * NCC * 512 + cc * 512
            return xtf[:, base:base + 512]

        def passA(sc):
            """Q^T for own (odd) q-blocks of this chunk, fp8 into qdr."""
            qp = psproj.tile([128, 256], F32, tag="proj")
            for cc in range(NCC):
                rhs = (
                    xts(sc, cc)
                    .rearrange("p (a b s) -> p a b s", a=2, b=2)[:, :, 1, :]
                )
                nc.tensor.matmul(
                    qp[64:128, :], wq_sb[:, cc * 64:(cc + 1) * 64], rhs,
                    start=(cc == 0), stop=(cc == NCC - 1),
                )
            nc.vector.tensor_scalar_add(
                qdr[64:128, :].rearrange("p (i q) -> p i q", i=2)[
                    :, 0, sc * 256:(sc + 1) * 256
                ],
                qp[64:128, :], bq_sb[64:128, :],
            )

        def passB(sc):
            """K^T (fp8, no bias) and V' from [wv|wk] stationary."""
            kvp = psproj.tile([128, 512], F32, tag="proj")
            for cc in range(NCC):
                nc.tensor.matmul(
                    kvp[:], wkv_sb[:, cc * 128:(cc + 1) * 128],
                    xts(sc, cc),
                    start=(cc == 0), stop=(cc == NCC - 1),
                )
            nc.vector.tensor_copy(
                kdr[64:128, :].rearrange("p (i m) -> p i m", i=2)[
                    :, 0, sc * 512:(sc + 1) * 512
                ],
                kvp[64:128, :],
            )
            vt = vtp.tile([128, 512], BF16)
            nc.vector.tensor_scalar_add(
                vt[0:64, :], kvp[0:64, :], bv_sb[:, :]
            )
            vp = psproj.tile([128, 256], BF16, tag="proj")
            for t in range(4):
                nc.tensor.transpose(
                    vp[:, t * 64:(t + 1) * 64],
                    vt[0:64, t * 128:(t + 1) * 128],
                    idn_sb[:],
                )
            nc.vector.tensor_copy(
                vs[:].rearrange("p (g e) -> p g e", g=NBLK)[
                    :, sc * 4:(sc + 1) * 4, 0:64
                ],
                vp[:].rearrange("p (g e) -> p g e", g=4),
            )
            if sc == 0:
                nc.vector.tensor_copy(
                    vsb[:].rearrange("p (g e) -> p g e", g=2)[:, :, 0:64],
                    vp[:].rearrange("p (g e) -> p g e", g=4)[:, 0:2, :],
                )
                # neutralize the j=0 dead slot 0 (pads = 0 there, 1 for j=1)
                nc.vector.tensor_scalar_mul(
                    vs[:, 0:80], vs[:, 0:80], pads_sb[:]
                )
                nc.vector.tensor_scalar_mul(
                    vsb[:, 0:65], vsb[:, 0:65], pads_sb[:]
                )

        def kslot(g):
            return kdr[64:128, :].rearrange("p (i m) -> p i m", i=2)[
                :, :, g * 128:(g + 1) * 128
            ]

        def qsl(c, off, w):
            return qdr[64:128, :].rearrange("p (i q) -> p i q", i=2)[
                :, :, c * 512 + off:c * 512 + off + w
            ]

        def attn_chunk(c, fillers):
            """All attention pairs of q-chunk c; fillers[p] emits projection
            work into the PE stream after pair p's score matmuls (p=-1:
            before the first pair)."""
            npairs = 4 * c + 4
            av = psav.tile([80, 512], F32, tag="av")
            pend = None
            fill = dict(fillers)
            if -1 in fill:
                fill.pop(-1)()

            def emit_av(args):
                av_, off, w, pt_, p = args
                if c == 0 and p == 0:
                    nc.tensor.matmul(
                        av_[0:65, 0:512], vsb[:, 0:65], pt_[:, 0:512],
                        start=True, stop=False,
                    )
                    nc.tensor.matmul(
                        av_[0:65, 0:512], vsb[:, 65:130], pt_[:, 512:1024],
                        start=False, stop=False,
                    )
                else:
                    nc.tensor.matmul(
                        av_[:, off:off + w],
                        vs[:].rearrange("p (g e) -> p g e", g=NBLK)[
                            :, 2 * p:2 * p + 2, :
                        ],
                        pt_[:].rearrange("p (i q) -> p i q", i=2)[
                            :, :, 512 - w:512
                        ],
                        start=(p == 0), stop=(p == npairs - 1),
                        perf_mode=DR,
                    )

            for p in range(npairs):
                g0, g1 = 2 * p, 2 * p + 1
                # pair geometry: both slots share offset and width
                off = 128 * max(0, p - (4 * c)) if p > 4 * c else 0
                w = 512 - off
                lo = 512 - w
                st = psst.tile([128, 1024], F32, tag="st")
                nc.tensor.matmul(
                    st[:, lo:512], kslot(g0), qsl(c, off, w),
                    start=True, stop=True, perf_mode=DR,
                )
                nc.tensor.matmul(
                    st[:, 512 + lo:1024], kslot(g1), qsl(c, off, w),
                    start=True, stop=True, perf_mode=DR,
                )
                if p in fill:
                    fill[p]()
                if pend is not None:
                    emit_av(pend)
                ptd = BF16 if (c == 0 and p == 0) else FP8
                pt = ptp.tile([128, 1024], ptd)
                if w == 512:
                    nc.scalar.activation(pt[:, 0:1024], st[:, 0:1024],
                                         EXPF, bias=0.0, scale=SCALE)
                else:
                    nc.scalar.activation(pt[:, lo:512], st[:, lo:512],
                                         EXPF, bias=0.0, scale=SCALE)
                    nc.scalar.activation(pt[:, 512 + lo:1024],
                                         st[:, 512 + lo:1024],
                                         EXPF, bias=0.0, scale=SCALE)
                if p >= npairs - 4:
                    # odd member of the last four pairs is causal-diagonal
                    nc.gpsimd.tensor_mul(
                        pt[:, 512 + lo:1024], pt[:, 512 + lo:1024],
                        mask_sb[:, 0:w]
                    )
                pend = (av, off, w, pt, p)
            emit_av(pend)
            oc = ocp.tile([65, 512], F32)
            nc.vector.tensor_copy(oc[:], av[0:65, :])
            nc.sync.dma_start(o[c, :, :], oc[:])

        for _rep in range(repeats):
            passA(0)
            passB(0)
            passA(1)
            attn_chunk(0, {
                1: lambda: passB(1),
                3: lambda: passA(2),
            })
            attn_chunk(1, {
                -1: lambda: passA(3),
                1: lambda: passB(2),
                3: lambda: passB(3),
                5: lambda: passA(4),
                7: lambda: passB(4),
            })
            attn_chunk(2, {
                -1: lambda: passA(5),
                2: lambda: passB(5),
                4: lambda: passA(6),
                6: lambda: passB(6),
                8: lambda: passA(7),
                10: lambda: passB(7),
            })
            attn_chunk(3, {})
        if dump:
            nc.sync.dma_start(
                okt[:, :],
                kdr[64:128, :].rearrange("p (i m) -> p i m", i=2)[:, 0, :],
            )
            nc.sync.dma_start(
                oqt[:, :],
                qdr[64:128, :].rearrange("p (i q) -> p i q", i=2)[:, 0, :],
            )
            nc.sync.dma_start(ovs[:, :], vs[:])

    nc.compile()
    return nc


def _get_nc():
    global _CACHED_NC
    if _CACHED_NC is None:
        _CACHED_NC = build_nc()
    return _CACHED_NC


def _host_inputs(x, wq, bq, wk, bk, wv, bv):
    bf = ml_dtypes.bfloat16
    # weights pre-arranged to the on-chip [p, (cc, m)] layout so the DMA
    # moves large contiguous runs
    wqr = np.ascontiguousarray(
        wq.reshape(NCC, 128, 64).transpose(1, 0, 2).reshape(128, NCC * 64)
    ).astype(bf)
    wkv = np.concatenate([wv, wk], axis=1)
    wkvr = np.ascontiguousarray(
        wkv.reshape(NCC, 128, 128).transpose(1, 0, 2).reshape(128, NCC * 128)
    ).astype(bf)
    bqc = bq[:, None].astype(np.float32)
    bvc = bv[:, None].astype(np.float32)
    tri = np.triu(np.ones((128, 128), np.float32))
    maska = np.concatenate([tri, np.ones((128, 384), np.float32)], axis=1).astype(bf)
    idnb = np.eye(64, dtype=np.float32).astype(bf)
    xbf = np.ascontiguousarray(x).astype(bf)

    in_maps = []
    for core in range(8):
        b, j = core // 2, core % 2
        if j == 0:
            xdev = np.concatenate(
                [np.zeros((128, DIN), bf), xbf[b][: SEQ - 128]], axis=0
            )
            ps = np.zeros((128, 1), np.float32)
        else:
            xdev = xbf[b]
            ps = np.ones((128, 1), np.float32)
        in_maps.append({
            "xt": np.ascontiguousarray(xdev.T),
            "wqr": wqr, "wkvr": wkvr, "bq": bqc, "bv": bvc,
            "pads": ps, "maska": maska, "idnb": idnb,
        })
    return in_maps


def _assemble(results):
    out = np.empty((4, SEQ, DOUT), np.float32)
    for core in range(8):
        b, j = core // 2, core % 2
        od = results[core]["o"]  # [NQC, 65, 512]
        for c in range(NQC):
            num = od[c, 0:64, :].astype(np.float64)
            den = od[c, 64, :].astype(np.float64)
            oc = (num / den).T.astype(np.float32)  # [512, 64]
            for t in range(4):
                r0 = (8 * c + 2 * t + j) * 128
                out[b, r0:r0 + 128] = oc[t * 128:(t + 1) * 128]
    return out


def kernel(x, wq, bq, wk, bk, wv, bv):
    x = np.asarray(x, dtype=np.float32)
    args = [np.asarray(a, dtype=np.float32) for a in (wq, bq, wk, bk, wv, bv)]
    nc = _get_nc()
    in_maps = _host_inputs(x, *args)
    br = run_bass_kernel_spmd(nc, in_maps, core_ids=list(range(8)))
    return _assemble(br.results)


# revision 7
# speedup vs baseline: 1.4929x; 1.0833x over previous
"""Trainium2 Bass kernel: causal attention (QKV projection + causal softmax + AV).

Problem: x[4, 4096, 768] fp32, per-head projections to d=64, full causal
attention per batch, output [4, 4096, 64] fp32.

Sharding: 8 cores = 4 batches x 2 parity groups. Core (b, j) computes the
output rows of batch b whose 128-row block index i satisfies i % 2 == j.
One uniform SPMD program: for j=0 cores the host shifts x down by one
128-row block (prepending zeros), which makes the causal structure of both
parities identical in device coordinates (device q-blocks are always the odd
blocks 1,3,...,31; k-slot g holds true block g-1 for j=0 and g for j=1; the
dead slot 0 of j=0 is neutralized by zeroing V' slot 0, so its exp(0)=1
weights contribute nothing to numerator or denominator).

Device pipeline per core:
  x^T arrives host-pre-transposed (plain DMA loads, no DMA-transpose).
  A short stream of dummy matmuls at t=0 keeps the tensor engine
  continuously busy so its p-state clock is fully ramped when real
  projections start.
  Projections per 512-row chunk: stationary [wq] produces Q^T on PSUM
  partitions 64-127; stationary [wv|wk] produces V^T (0-63) and K^T (64-127).
  Q^T (+bq) and K^T (bk dropped: softmax is invariant to score offsets that
  are constant along k) are written as fp8e4 into zero-padded DoubleRow
  buffers [64, 2, cols] whose second contraction-tile group is zeroed once.
  Attention runs as one flat pipeline of slot pairs across all q-chunks
  (chunk 0 split into two 256-column halves to shorten the startup
  dependency), with projection work for later chunks emitted between pairs
  and each pair's AV lagging one pair behind its scores so the tensor
  engine never waits on the exp.
  Scores: one fp8 DoubleRow matmul per k-slot (0.5 cycles/row), the pair's
  slots packed tail/head around the tile midpoint so each pair is a single
  exact-width exp on ACT. The two slots of a pair share width and column
  range, so AV is one DoubleRow matmul per pair over fp8 P and fp8 V'
  (the pair as the two contraction-tile groups, 80-byte slot stride for
  the dual-fp8 16-byte alignment rule), accumulating into a [80, 512] PSUM
  tile whose row 64 is the softmax denominator. Chunk-0a pair-0 (rows that
  attend very few keys, where fp8 V error would not average out) uses bf16
  P/V'. Host divides and transposes.
"""

import numpy as np
import ml_dtypes
from contextlib import ExitStack

import concourse.bass as bass
import concourse.mybir as mybir
import concourse.tile as tile
from concourse import bacc
from concourse.bass_utils import run_bass_kernel_spmd

F32 = mybir.dt.float32
BF16 = mybir.dt.bfloat16
FP8 = mybir.dt.float8e4

SEQ = 4096
DIN = 768
DOUT = 64
NCC = DIN // 128          # 6 contraction chunks
NSC = SEQ // 512          # 8 seq chunks (projection granularity)
NBLK = SEQ // 128         # 32 k-slots
NQC = 4                   # q chunks of 512 local columns (2048 own q rows)
NWARM = 16                # PE p-state warmup matmuls
SCALE = 1.0 / 8.0
EXPF = mybir.ActivationFunctionType.Exp
DR = mybir.MatmulPerfMode.DoubleRow

_CACHED_NC = None


def build_nc(dump=False, repeats=1):
    nc = bacc.Bacc("TRN2", target_bir_lowering=False, debug=False)

    xt = nc.dram_tensor("xt", [DIN, SEQ], BF16, kind="ExternalInput")
    wqr = nc.dram_tensor("wqr", [128, NCC * 64], BF16, kind="ExternalInput")
    wkvr = nc.dram_tensor("wkvr", [128, NCC * 128], BF16, kind="ExternalInput")
    bq = nc.dram_tensor("bq", [64, 1], F32, kind="ExternalInput")
    bv = nc.dram_tensor("bv", [64, 1], F32, kind="ExternalInput")
    pads = nc.dram_tensor("pads", [128, 1], F32, kind="ExternalInput")   # 1 / 0
    maska = nc.dram_tensor("maska", [128, 512], BF16, kind="ExternalInput")
    idnb = nc.dram_tensor("idnb", [64, 64], BF16, kind="ExternalInput")
    o = nc.dram_tensor("o", [NQC, 65, 512], F32, kind="ExternalOutput")

    with tile.TileContext(nc) as tc, ExitStack() as ctx:
        cpool = ctx.enter_context(tc.tile_pool(name="const", bufs=1))
        vtp = ctx.enter_context(tc.tile_pool(name="vt", bufs=2))
        ptp = ctx.enter_context(tc.tile_pool(name="pt", bufs=3))
        ocp = ctx.enter_context(tc.tile_pool(name="oc", bufs=2))
        psproj = ctx.enter_context(tc.tile_pool(name="psproj", bufs=2, space="PSUM"))
        psst = ctx.enter_context(tc.tile_pool(name="psst", bufs=2, space="PSUM"))
        psav = ctx.enter_context(tc.tile_pool(name="psav", bufs=2, space="PSUM"))

        wq_sb = cpool.tile([128, NCC * 64], BF16)
        wkv_sb = cpool.tile([128, NCC * 128], BF16)
        bq_sb = cpool.tile([128, 1], F32)     # rows 64:128 hold bq
        bv_sb = cpool.tile([64, 1], F32)
        pads_sb = cpool.tile([128, 1], F32)
        mask_sb = cpool.tile([128, 512], BF16)
        idn_sb = cpool.tile([64, 64], BF16)
        warm = cpool.tile([128, 256], BF16)
        xtf = cpool.tile([128, NSC * NCC * 512], BF16)  # x^T, [p, (sc, cc, s)]
        qdr = cpool.tile([128, 2 * 2048], FP8)  # Q^T fp8, rows 64:128, [(i, q)]
        kdr = cpool.tile([128, 2 * 4096], FP8)  # K^T fp8, rows 64:128, [(i, m)]
        vs = cpool.tile([128, NBLK * 80], FP8)  # V' = [V | 1 | 0pad] per k-slot
        # (80-wide slots: dual-fp8 ldweights needs a 16-byte-aligned
        # stride between the two contraction-tile groups)
        vsb = cpool.tile([128, 2 * 65], BF16)   # bf16 V' for slots 0,1

        # PE p-state warmup: dummy matmuls on a zeroed tile keep the tensor
        # engine busy from t~0 so the clock is ramped when real work arrives
        nc.vector.memset(warm[:], 0.0)
        for _ in range(NWARM):
            wp = psproj.tile([128, 256], F32, tag="proj")
            nc.tensor.matmul(wp[:], warm[:, 0:128], warm[:], start=True, stop=True)

        def loadx(sc):
            nc.sync.dma_start(
                xtf[:, sc * NCC * 512:(sc + 1) * NCC * 512]
                .rearrange("p (cc s) -> p cc s", cc=NCC),
                xt.rearrange("(cc p) s -> p cc s", p=128)[
                    :, :, sc * 512:(sc + 1) * 512
                ],
            )

        loadx(0)
        nc.sync.dma_start(wq_sb[:], wqr[:, :])
        nc.sync.dma_start(bq_sb[64:128, :], bq[:, :])
        nc.sync.dma_start(wkv_sb[:], wkvr[:, :])
        nc.sync.dma_start(bv_sb[:], bv[:, :])
        nc.sync.dma_start(idn_sb[:], idnb[:, :])
        nc.sync.dma_start(mask_sb[:], maska[:, :])
        nc.sync.dma_start(pads_sb[:], pads[:, :])
        for sc in range(1, NSC):
            loadx(sc)

        # ones column + zero pad of V'
        nc.vector.memset(
            vs[:].rearrange("p (g e) -> p g e", g=NBLK)[:, :, 64:65], 1.0
        )
        nc.vector.memset(
            vs[:].rearrange("p (g e) -> p g e", g=NBLK)[:, :, 65:80], 0.0
        )
        nc.vector.memset(
            vsb[:].rearrange("p (g e) -> p g e", g=2)[:, :, 64:65], 1.0
        )
        # zero the second DoubleRow contraction-tile group of Q^T/K^T (both
        # sides, guarding against NaN garbage multiplying the other's zeros);
        # ordered so the regions attention needs first are zeroed first
        nc.gpsimd.memset(qdr[64:128, 2048:2048 + 512], 0.0)
        nc.gpsimd.memset(kdr[64:128, 4096:4096 + 1024], 0.0)
        nc.gpsimd.memset(kdr[64:128, 4096 + 1024:8192], 0.0)
        nc.gpsimd.memset(qdr[64:128, 2048 + 512:4096], 0.0)

        def xts(sc, cc):
            base = sc * NCC * 512 + cc * 512
            return xtf[:, base:base + 512]

        def passA(sc):
            """Q^T for own (odd) q-blocks of this chunk, fp8 into qdr."""
            qp = psproj.tile([128, 256], F32, tag="proj")
            for cc in range(NCC):
                rhs = (
                    xts(sc, cc)
                    .rearrange("p (a b s) -> p a b s", a=2, b=2)[:, :, 1, :]
                )
                nc.tensor.matmul(
                    qp[64:128, :], wq_sb[:, cc * 64:(cc + 1) * 64], rhs,
                    start=(cc == 0), stop=(cc == NCC - 1),
                )
            nc.vector.tensor_scalar_add(
                qdr[64:128, :].rearrange("p (i q) -> p i q", i=2)[
                    :, 0, sc * 256:(sc + 1) * 256
                ],
                qp[64:128, :], bq_sb[64:128, :],
            )

        vt_pend = {}

        def passB_kv(sc):
            """K^T (fp8, no bias) into kdr; V^T (+bias) into a bf16 staging
            tile (transposed into V' by passB_vt)."""
            kvp = psproj.tile([128, 512], F32, tag="proj")
            for cc in range(NCC):
                nc.tensor.matmul(
                    kvp[:], wkv_sb[:, cc * 128:(cc + 1) * 128],
                    xts(sc, cc),
                    start=(cc == 0), stop=(cc == NCC - 1),
                )
            vt = vtp.tile([128, 512], BF16)
            nc.vector.tensor_scalar_add(
                vt[0:64, :], kvp[0:64, :], bv_sb[:, :]
            )
            nc.vector.tensor_copy(
                kdr[64:128, :].rearrange("p (i m) -> p i m", i=2)[
                    :, 0, sc * 512:(sc + 1) * 512
                ],
                kvp[64:128, :],
            )
            vt_pend[sc] = vt

        def passB_vt(sc):
            """PE-transpose V^T chunk into fp8 V' slots."""
            vt = vt_pend.pop(sc)
            vp = psproj.tile([128, 256], BF16, tag="proj")
            for t in range(4):
                nc.tensor.transpose(
                    vp[:, t * 64:(t + 1) * 64],
                    vt[0:64, t * 128:(t + 1) * 128],
                    idn_sb[:],
                )
            nc.vector.tensor_copy(
                vs[:].rearrange("p (g e) -> p g e", g=NBLK)[
                    :, sc * 4:(sc + 1) * 4, 0:64
                ],
                vp[:].rearrange("p (g e) -> p g e", g=4),
            )
            if sc == 0:
                nc.vector.tensor_copy(
                    vsb[:].rearrange("p (g e) -> p g e", g=2)[:, :, 0:64],
                    vp[:].rearrange("p (g e) -> p g e", g=4)[:, 0:2, :],
                )
                # neutralize the j=0 dead slot 0 (pads = 0 there, 1 for j=1)
                nc.vector.tensor_scalar_mul(
                    vs[:, 0:80], vs[:, 0:80], pads_sb[:]
                )
                nc.vector.tensor_scalar_mul(
                    vsb[:, 0:65], vsb[:, 0:65], pads_sb[:]
                )

        def passB(sc):
            passB_kv(sc)
            passB_vt(sc)

        def kslot(g):
            return kdr[64:128, :].rearrange("p (i m) -> p i m", i=2)[
                :, :, g * 128:(g + 1) * 128
            ]

        def qsl(qlo, w):
            return qdr[64:128, :].rearrange("p (i q) -> p i q", i=2)[
                :, :, qlo:qlo + w
            ]

        # flat pair pipeline: chunk 0 split into 256-col halves 0a/0b, then
        # chunks 1..3. Each pair: two k-slots sharing offset and width.
        #   (key, H, qbase, avw, oslice, pairs[(g0, off, w, masked)])
        chunks = [
            ("0a", 256, 0, 256, (0, 0, 256), [
                (0, 0, 256, True), (2, 128, 128, True)]),
            ("0b", 256, 256, 256, (0, 256, 512), [
                (0, 0, 256, False), (2, 0, 256, False),
                (4, 0, 256, True), (6, 128, 128, True)]),
        ]
        for c in range(1, NQC):
            prs = []
            for p in range(4 * c + 4):
                off = 128 * max(0, p - 4 * c)
                prs.append((2 * p, off, 512 - off, p >= 4 * c))
            chunks.append((str(c), 512, c * 512, 512, (c, 0, 512), prs))

        # global pair index -> projection emitters (PE-stream fillers)
        fillers = {
            0: [lambda: passA(1)],
            1: [lambda: passB_kv(1)],
            2: [lambda: passB_vt(1)],
            3: [lambda: passA(2)],
            4: [lambda: passA(3)],
            6: [lambda: passB_kv(2)],
            7: [lambda: passB_vt(2)],
            8: [lambda: passB_kv(3)],
            9: [lambda: passB_vt(3)],
            10: [lambda: passA(4)],
            11: [lambda: passA(5)],
            14: [lambda: passB_kv(4)],
            16: [lambda: passB_vt(4)],
            18: [lambda: passB_kv(5)],
            19: [lambda: passB_vt(5)],
            20: [lambda: passA(6)],
            21: [lambda: passA(7)],
            24: [lambda: passB_kv(6)],
            28: [lambda: passB_vt(6)],
            30: [lambda: passB_kv(7)],
            32: [lambda: passB_vt(7)],
        }

        def emit_av(a):
            av_, off, w, pt_, H, pidx, npairs, bf16, out, g0 = a
            if bf16:
                nc.tensor.matmul(
                    av_[0:65, 0:256], vsb[:, 0:65], pt_[:, 0:256],
                    start=True, stop=False,
                )
                nc.tensor.matmul(
                    av_[0:65, 0:256], vsb[:, 65:130], pt_[:, 256:512],
                    start=False, stop=False,
                )
            else:
                nc.tensor.matmul(
                    av_[:, off:off + w],
                    vs[:].rearrange("p (g e) -> p g e", g=NBLK)[
                        :, g0:g0 + 2, :
                    ],
                    pt_[:, H - w:H + w].rearrange("p (i q) -> p i q", i=2),
                    start=(pidx == 0), stop=(pidx == npairs - 1),
                    perf_mode=DR,
                )
            if out is not None:
                c, lo, hi = out
                oc = ocp.tile([65, hi - lo], F32)
                nc.vector.tensor_copy(oc[:], av_[0:65, 0:hi - lo])
                nc.sync.dma_start(o[c, :, lo:hi], oc[:])

        def emit_attention():
            gi = 0
            pend = None
            for key, H, qbase, avw, oslice, prs in chunks:
                av = psav.tile([80, avw], F32, tag="av")
                npairs = len(prs)
                for pidx, (g0, off, w, masked) in enumerate(prs):
                    st = psst.tile([128, 2 * H], F32, tag="st")
                    nc.tensor.matmul(
                        st[:, H - w:H], kslot(g0), qsl(qbase + off, w),
                        start=True, stop=True, perf_mode=DR,
                    )
                    nc.tensor.matmul(
                        st[:, H:H + w], kslot(g0 + 1), qsl(qbase + off, w),
                        start=True, stop=True, perf_mode=DR,
                    )
                    for fn in fillers.get(gi, ()):
                        fn()
                    if pend is not None:
                        emit_av(pend)
                        pend = None
                    bf16 = (key == "0a" and pidx == 0)
                    pt = ptp.tile([128, 2 * H], BF16 if bf16 else FP8)
                    nc.scalar.activation(pt[:, H - w:H + w], st[:, H - w:H + w],
                                         EXPF, bias=0.0, scale=SCALE)
                    if masked:
                        nc.gpsimd.tensor_mul(
                            pt[:, H:H + w], pt[:, H:H + w], mask_sb[:, 0:w]
                        )
                    pend = (av, off, w, pt, H, pidx, npairs, bf16,
                            oslice if pidx == npairs - 1 else None, g0)
                    gi += 1
            emit_av(pend)

        for _rep in range(repeats):
            passA(0)
            passB(0)
            emit_attention()

    nc.compile()
    return nc


def _get_nc():
    global _CACHED_NC
    if _CACHED_NC is None:
        _CACHED_NC = build_nc()
    return _CACHED_NC


def _host_inputs(x, wq, bq, wk, bk, wv, bv):
    bf = ml_dtypes.bfloat16
    # weights pre-arranged to the on-chip [p, (cc, m)] layout so the DMA
    # moves large contiguous runs
    wqr = np.ascontiguousarray(
        wq.reshape(NCC, 128, 64).transpose(1, 0, 2).reshape(128, NCC * 64)
    ).astype(bf)
    wkv = np.concatenate([wv, wk], axis=1)
    wkvr = np.ascontiguousarray(
        wkv.reshape(NCC, 128, 128).transpose(1, 0, 2).reshape(128, NCC * 128)
    ).astype(bf)
    bqc = bq[:, None].astype(np.float32)
    bvc = bv[:, None].astype(np.float32)
    tri = np.triu(np.ones((128, 128), np.float32))
    maska = np.concatenate([tri, np.ones((128, 384), np.float32)], axis=1).astype(bf)
    idnb = np.eye(64, dtype=np.float32).astype(bf)
    xbf = np.ascontiguousarray(x).astype(bf)

    in_maps = []
    for core in range(8):
        b, j = core // 2, core % 2
        if j == 0:
            xdev = np.concatenate(
                [np.zeros((128, DIN), bf), xbf[b][: SEQ - 128]], axis=0
            )
            ps = np.zeros((128, 1), np.float32)
        else:
            xdev = xbf[b]
            ps = np.ones((128, 1), np.float32)
        in_maps.append({
            "xt": np.ascontiguousarray(xdev.T),
            "wqr": wqr, "wkvr": wkvr, "bq": bqc, "bv": bvc,
            "pads": ps, "maska": maska, "idnb": idnb,
        })
    return in_maps


def _assemble(results):
    out = np.empty((4, SEQ, DOUT), np.float32)
    for core in range(8):
        b, j = core // 2, core % 2
        od = results[core]["o"]  # [NQC, 65, 512]
        for c in range(NQC):
            num = od[c, 0:64, :].astype(np.float64)
            den = od[c, 64, :].astype(np.float64)
            oc = (num / den).T.astype(np.float32)  # [512, 64]
            for t in range(4):
                r0 = (8 * c + 2 * t + j) * 128
                out[b, r0:r0 + 128] = oc[t * 128:(t + 1) * 128]
    return out


def kernel(x, wq, bq, wk, bk, wv, bv):
    x = np.asarray(x, dtype=np.float32)
    args = [np.asarray(a, dtype=np.float32) for a in (wq, bq, wk, bk, wv, bv)]
    nc = _get_nc()
    in_maps = _host_inputs(x, *args)
    br = run_bass_kernel_spmd(nc, in_maps, core_ids=list(range(8)))
    return _assemble(br.results)
